# revision 19
# baseline (speedup 1.0000x reference)
"""Trainium2 Bass kernel for nn_MultiHeadAttention_60559038873660.

Reference math (faithful to the source bug: attention is contracted with the
projected K, not V, so v/Wv are dead inputs):
    qp = q @ Wq.T ; kp = k @ Wk.T
    head split via reshape(b, l, 64, 16): head n takes strided columns {d*16+n}
    S = Qh @ Kh.T / 8 ; A = softmax(S, axis=m) ; X = A @ Kh ; out = X @ Wo.T

Strategy:
  - Host-side: permute weight rows/cols head-major so each head is a contiguous
    64-column block; pre-transpose q/k/weights into the layouts the TensorE
    wants (contraction on partitions).
  - 8 cores = 2 batches x 4 head-groups (4 heads each).  Each core computes its
    4 heads' attention plus a partial output projection; the host sums the 4
    partials per batch (tensor-parallel row-split reduction).
  - The kernel is a race between ScalarE exp (~145us of ACTIVATEs, the only
    exp engine) and TensorE (~170us of matmuls).  Everything else hides:
      * each input tile is DMA-loaded once and projected for BOTH head groups
      * attention starts right after k-tile 0 + q-tile 0 are projected; the
        remaining projections are need-ordered fillers paced through
        attention, out-projection strips pace through attention g1
      * Kh transposes (khT -> khp) run on the DMA XBAR (dma_start_transpose),
        not the PE
      * S-matmul head pairs run concurrently in the PE array (row groups)
      * X matmuls carry a fused ones-column so softmax denominators fall out
        of the X^T accumulation for free; the reciprocal+broadcast round-trip
        is deferred one strip so its DMA latency never blocks any queue
"""

import contextlib
import ctypes
import os
import sys
import types

import numpy as np

import concourse.bacc as bacc
import concourse.tile as tile
from concourse import mybir
from concourse.bass import ds, ts
from concourse.bass_utils import run_bass_kernel_spmd


def _install_ntff_hook():
    """Provide antenv.axon_hooks if the image lacks it, wiring NTFF
    profiling straight into libaxon_pjrt.so (same ABI trn_boot uses)."""
    try:
        import antenv.axon_hooks  # noqa: F401
        return
    except ImportError:
        pass
    mod = types.ModuleType("antenv.axon_hooks")
    holder = [None]
    mod.set_axon_ntff_profile_hook = lambda h: holder.__setitem__(0, h)
    mod.get_axon_ntff_profile_hook = lambda: holder[0]
    sys.modules["antenv.axon_hooks"] = mod
    try:
        import antenv
        antenv.axon_hooks = mod
    except ImportError:
        pass

    so_path = "/opt/axon/libaxon_pjrt.so"
    if not os.path.exists(so_path):
        return
    lib = ctypes.CDLL(so_path)
    if not hasattr(lib, "axon_start_nrt_profile"):
        return
    lib.axon_start_nrt_profile.argtypes = [ctypes.POINTER(ctypes.c_int64), ctypes.c_size_t]
    lib.axon_start_nrt_profile.restype = ctypes.c_int64
    lib.axon_stop_nrt_profile.argtypes = [ctypes.c_char_p]
    lib.axon_stop_nrt_profile.restype = ctypes.c_int64

    @contextlib.contextmanager
    def _hook(output_dir, device_ids):
        import jax
        jax.devices()
        if device_ids:
            ids = (ctypes.c_int64 * len(device_ids))(*device_ids)
            rc = lib.axon_start_nrt_profile(ids, len(device_ids))
        else:
            rc = lib.axon_start_nrt_profile(None, 0)
        if rc != 0:
            raise RuntimeError(f"axon_start_nrt_profile rc={rc}")
        try:
            yield
        finally:
            n = lib.axon_stop_nrt_profile(str(output_dir).encode())
            print(f"profile: {n} file(s) written to {output_dir}", file=sys.stderr)

    mod.set_axon_ntff_profile_hook(_hook)


_install_ntff_hook()

f32 = mybir.dt.float32
bf16 = mybir.dt.bfloat16
Exp = mybir.ActivationFunctionType.Exp

P = 128
DIM = 1024
NH = 16
HD = 64
HPC = 4          # heads per core
CW = HPC * HD    # 256 channel columns per core
CH = HD + 1      # head channels + ones column
G = CW // P      # 2 channel groups of 128
KC = DIM // P    # 8 contraction chunks for projections
JT = DIM // 512  # out-projection j tiles

_cache = {}


def _build(L, M):
    NT = 512                  # matmul moving-dim tile / l-strip size
    LT = L // NT              # 4 q tiles
    MT = M // NT              # 4 k tiles
    MG = M // P               # 16 m chunks for attention
    L5 = L // NT              # 4 attention l-strips per group
    LC = L // P               # 16 out-projection row chunks

    nc = bacc.Bacc()
    qT = nc.declare_dram_parameter("qT", [DIM, L], bf16, isOutput=False)
    kT = nc.declare_dram_parameter("kT", [DIM, M], bf16, isOutput=False)
    wqT = nc.declare_dram_parameter("wqT", [DIM, CW], bf16, isOutput=False)
    wkT = nc.declare_dram_parameter("wkT", [DIM, CW], bf16, isOutput=False)
    woT = nc.declare_dram_parameter("woT", [CW, DIM], bf16, isOutput=False)
    out = nc.declare_dram_parameter("out", [L, DIM], f32, isOutput=True)

    with tile.TileContext(nc) as tc:
        with (
            tc.tile_pool(name="singles", bufs=1) as singles,
            tc.tile_pool(name="io", bufs=8) as io,
            tc.tile_pool(name="es", bufs=4) as es_pool,
            tc.tile_pool(name="opool", bufs=3) as opool,
            tc.tile_pool(name="dstp", bufs=4) as dstp,
            tc.tile_pool(name="pbp", bufs=4) as pbp,
        ):
            # exp table preload: a tiny dummy activation at the head of the
            # Scalar queue pulls the ~2.7us ACT_TABLE_LOAD into the prologue
            dum = singles.tile([1, 16], f32)
            dum2 = singles.tile([1, 16], f32)
            nc.vector.memset(dum, 0.0)
            nc.scalar.activation(dum2, dum, Exp, scale=1.0)

            wk_sb = singles.tile([P, KC, CW], bf16)
            nc.sync.dma_start(wk_sb, wkT.rearrange("(kc p) c -> p kc c", p=P))
            wq_sb = singles.tile([P, KC, CW], bf16)
            wo_sb = singles.tile([P, G, DIM], bf16)

            qhT = singles.tile([P, G, L], bf16)
            khT = singles.tile([P, G, M], bf16)
            # khp: per (m-chunk, group) the two heads' Kh blocks, each with a
            # trailing ones column: [hA(64) | 1s | hB(64) | 1s]
            khp = singles.tile([P, MG, G, 2 * CH], bf16)
            xu = singles.tile([P, G, L], bf16)

            ones_sb = singles.tile([P, 1], f32)
            nc.vector.memset(ones_sb, 1.0)
            for mg in range(MG):
                for hh in range(2):
                    nc.vector.tensor_copy(
                        khp[:, mg, :, hh * CH + HD:hh * CH + CH],
                        ones_sb[:, None, :].to_broadcast([P, G, 1]))

            with (
                tc.tile_pool(name="psP", bufs=2, space="PSUM") as psP,
                tc.tile_pool(name="psS", bufs=2, space="PSUM") as psS,
                tc.tile_pool(name="psX", bufs=2, space="PSUM") as psX,
            ):
                # ---- projection building blocks (one input load serves both
                # head groups) ----
                def load_tile(src_ap, tt):
                    in_t = io.tile([P, KC, NT], bf16, tag="io")
                    nc.sync.dma_start(
                        in_t, src_ap[:, ts(tt, NT)].rearrange("(kc p) l -> p kc l", p=P))
                    return in_t

                def proj_half(dst, w_sb, in_t, tt, g, half, st):
                    if half == 0:
                        ps = psP.tile([P, NT], f32, tag="ps")
                        st["ps"] = ps
                        for kc in range(KC // 2):
                            nc.tensor.matmul(ps, lhsT=w_sb[:, kc, ts(g, P)],
                                             rhs=in_t[:, kc],
                                             start=(kc == 0), stop=False)
                    else:
                        ps = st["ps"]
                        for kc in range(KC // 2, KC):
                            nc.tensor.matmul(ps, lhsT=w_sb[:, kc, ts(g, P)],
                                             rhs=in_t[:, kc],
                                             start=False, stop=(kc == KC - 1))
                        nc.vector.tensor_copy(dst[:, g, ts(tt, NT)], ps)

                from concourse.masks import make_identity
                ident = singles.tile([P, P], bf16)
                make_identity(nc, ident)

                def ktrans(mt, g):
                    # khT [c, m] -> khp [m, c], one k-tile (4 m-chunks) per call
                    for mc in range(4 * mt, 4 * mt + 4):
                        tr = psP.tile([P, P], bf16, tag="ps")
                        nc.tensor.transpose(tr, khT[:, g, ts(mc, P)], ident)
                        for hh in range(2):
                            nc.vector.tensor_copy(
                                khp[:, mc, g, hh * CH:hh * CH + HD],
                                tr[:, ts(hh, HD)])

                # ---- out-projection strip (row chunk lc, column tile jt) ----
                def outproj(lc, jt):
                    po = psP.tile([P, NT], f32, tag="ps")
                    for cc in range(G):
                        nc.tensor.matmul(po, lhsT=xu[:, cc, ts(lc, P)],
                                         rhs=wo_sb[:, cc, ts(jt, NT)],
                                         start=(cc == 0), stop=(cc == G - 1))
                    ot = opool.tile([P, NT], f32, tag="ot")
                    nc.vector.tensor_copy(ot, po)
                    nc.sync.dma_start(out[ts(lc, P), ts(jt, NT)], ot)

                # ---- DMAs: k-side on the sync queue, q-side on the scalar
                # queue so the two streams load concurrently and the prologue
                # projections start ~4us in
                def load_tile_q(src_ap, tt):
                    in_t = io.tile([P, KC, NT], bf16, tag="io")
                    nc.scalar.dma_start(
                        in_t, src_ap[:, ts(tt, NT)].rearrange("(kc p) l -> p kc l", p=P))
                    return in_t

                kt0 = load_tile(kT, 0)
                nc.scalar.dma_start(wq_sb, wqT.rearrange("(kc p) c -> p kc c", p=P))
                qt0 = load_tile_q(qT, 0)
                kt1 = load_tile(kT, 1)
                kt2 = load_tile(kT, 2)
                kt3 = load_tile(kT, 3)
                qt1 = load_tile_q(qT, 1)
                qt2 = load_tile_q(qT, 2)
                qt3 = load_tile_q(qT, 3)
                nc.scalar.dma_start(wo_sb, woT.rearrange("(g p) j -> p g j", p=P))

                # ---- prologue: the minimum compute before attention starts
                st = {}
                proj_half(khT, wk_sb, kt0, 0, 0, 0, st)
                proj_half(khT, wk_sb, kt0, 0, 0, 1, st)
                ktrans(0, 0)
                st = {}
                proj_half(qhT, wq_sb, qt0, 0, 0, 0, st)
                proj_half(qhT, wq_sb, qt0, 0, 0, 1, st)

                # ---- filler atoms, ordered by first use ----
                fillers = []

                def add_proj_atoms(dst, w_sb, in_t, tt, g):
                    st = {}
                    fillers.append(lambda st=st: proj_half(dst, w_sb, in_t, tt, g, 0, st))
                    fillers.append(lambda st=st: proj_half(dst, w_sb, in_t, tt, g, 1, st))

                add_proj_atoms(khT, wk_sb, kt1, 1, 0)
                fillers.append(lambda: ktrans(1, 0))
                add_proj_atoms(khT, wk_sb, kt2, 2, 0)
                fillers.append(lambda: ktrans(2, 0))
                add_proj_atoms(khT, wk_sb, kt3, 3, 0)
                fillers.append(lambda: ktrans(3, 0))
                add_proj_atoms(qhT, wq_sb, qt1, 1, 0)
                add_proj_atoms(khT, wk_sb, kt0, 0, 1)
                fillers.append(lambda: ktrans(0, 1))
                add_proj_atoms(khT, wk_sb, kt1, 1, 1)
                fillers.append(lambda: ktrans(1, 1))
                add_proj_atoms(khT, wk_sb, kt2, 2, 1)
                fillers.append(lambda: ktrans(2, 1))
                add_proj_atoms(khT, wk_sb, kt3, 3, 1)
                fillers.append(lambda: ktrans(3, 1))
                add_proj_atoms(qhT, wq_sb, qt2, 2, 0)
                add_proj_atoms(qhT, wq_sb, qt3, 3, 0)
                add_proj_atoms(qhT, wq_sb, qt0, 0, 1)
                add_proj_atoms(qhT, wq_sb, qt1, 1, 1)
                add_proj_atoms(qhT, wq_sb, qt2, 2, 1)
                add_proj_atoms(qhT, wq_sb, qt3, 3, 1)

                # ---- attention ----
                def emit_sp(g, lsl, mc):
                    sps = psS.tile([P, 2 * NT], f32, tag="s")
                    nc.tensor.matmul(sps[:, 0:NT],
                                     lhsT=khT[0:HD, g, ts(mc, P)],
                                     rhs=qhT[0:HD, g, lsl],
                                     start=True, stop=True)
                    nc.tensor.matmul(sps[:, NT:2 * NT],
                                     lhsT=khT[HD:P, g, ts(mc, P)],
                                     rhs=qhT[HD:P, g, lsl],
                                     start=True, stop=True)
                    return sps

                def normalize_rest(g, lsl, dbcs):
                    # deferred one strip: the partition_broadcast issued at the
                    # end of strip (g,l5) has long completed, so the reciprocal
                    # never waits inside the Vector queue
                    def atom():
                        for hh in range(2):
                            dbc = dbcs[hh]
                            nc.vector.reciprocal(dbc[ts(hh, HD)], dbc[ts(hh, HD)])
                            nc.vector.tensor_mul(xu[ts(hh, HD), g, lsl],
                                                 xu[ts(hh, HD), g, lsl],
                                                 dbc[ts(hh, HD)])
                    return atom

                for g in range(G):
                    for l5 in range(L5):
                        lsl = ts(l5, NT)

                        xpsA = psX.tile([CH, NT], f32, tag="x")
                        xpsB = psX.tile([CH, NT], f32, tag="x")
                        sq = [emit_sp(g, lsl, 0)]
                        if MG > 1:
                            sq.append(emit_sp(g, lsl, 1))
                        for mc in range(MG):
                            if mc + 2 < MG:
                                sq.append(emit_sp(g, lsl, mc + 2))
                            es = es_pool.tile([P, 2 * NT], bf16, tag="es")
                            nc.scalar.activation(es, sq.pop(0), Exp, scale=0.125)
                            if fillers:
                                fillers.pop(0)()
                            nc.tensor.matmul(xpsA, lhsT=khp[:, mc, g, 0:CH],
                                             rhs=es[:, 0:NT],
                                             start=(mc == 0), stop=(mc == MG - 1))
                            nc.tensor.matmul(xpsB, lhsT=khp[:, mc, g, CH:2 * CH],
                                             rhs=es[:, NT:2 * NT],
                                             start=(mc == 0), stop=(mc == MG - 1))

                        dbcs = []
                        for hh, xps in ((0, xpsA), (1, xpsB)):
                            nc.vector.tensor_copy(xu[ts(hh, HD), g, lsl], xps[0:HD])
                            dstg = dstp.tile([1, NT], f32, tag="dst")
                            nc.vector.tensor_copy(dstg, xps[HD:CH])
                            dbc = pbp.tile([P, NT], f32, tag="pb")
                            nc.gpsimd.partition_broadcast(dbc, dstg)
                            dbcs.append(dbc)
                        # reciprocal + multiply deferred one strip
                        fillers.insert(0, normalize_rest(g, lsl, dbcs))

                        if g == 1:
                            # out-projection for this strip becomes filler work
                            # paced through the next strip; it sits after this
                            # strip's normalize atom, so it reads normalized xu
                            for lc in range(l5 * (NT // P), (l5 + 1) * (NT // P)):
                                for jt in range(JT):
                                    fillers.append(
                                        lambda lc=lc, jt=jt: outproj(lc, jt))

                while fillers:
                    fillers.pop(0)()

    nc.finalize()
    return nc


def _get_nc(L, M):
    key = (L, M)
    if key not in _cache:
        _cache[key] = _build(L, M)
    return _cache[key]


# head-major channel permutation: new channel c = h*64+d <- original column d*16+h
_PERM = np.array([(c % HD) * NH + c // HD for c in range(DIM)])

last_exec_time_ns = None
last_results = None


def kernel(q, k, v, Wq, Wk, Wv, Wo):  # noqa: ARG001 - v/Wv dead in reference
    global last_exec_time_ns, last_results
    q = np.asarray(q, np.float32)
    k = np.asarray(k, np.float32)
    Wq = np.asarray(Wq, np.float32)
    Wk = np.asarray(Wk, np.float32)
    Wo = np.asarray(Wo, np.float32)
    B, L, _ = q.shape
    M = k.shape[1]

    import ml_dtypes
    bf = ml_dtypes.bfloat16
    Wq_p = Wq[_PERM]            # (1024, 1024) head-major rows
    Wk_p = Wk[_PERM]
    WoT_p = Wo[:, _PERM].T      # (1024 c, 1024 j)

    qT = [np.ascontiguousarray(q[b].T).astype(bf) for b in range(B)]
    kT = [np.ascontiguousarray(k[b].T).astype(bf) for b in range(B)]
    wqT = [np.ascontiguousarray(Wq_p[hg * CW:(hg + 1) * CW, :].T).astype(bf) for hg in range(4)]
    wkT = [np.ascontiguousarray(Wk_p[hg * CW:(hg + 1) * CW, :].T).astype(bf) for hg in range(4)]
    woT = [np.ascontiguousarray(WoT_p[hg * CW:(hg + 1) * CW, :]).astype(bf) for hg in range(4)]

    in_maps = []
    for core in range(8):
        b, hg = divmod(core, 4)
        in_maps.append({"qT": qT[b], "kT": kT[b], "wqT": wqT[hg],
                        "wkT": wkT[hg], "woT": woT[hg]})

    nc = _get_nc(L, M)
    trace = bool(int(os.environ.get("MHA_TRACE", "0")))
    res = run_bass_kernel_spmd(nc, in_maps, core_ids=list(range(8)), trace=trace)
    last_results = res
    last_exec_time_ns = res.exec_time_ns

    out = np.zeros((B, L, DIM), np.float32)
    for core in range(8):
        b = core // 4
        out[b] += res.results[core]["out"]
    return out


# revision 26
# speedup vs baseline: 1.1714x; 1.1714x over previous
"""Trainium2 Bass kernel for nn_MultiHeadAttention_60559038873660.

Reference math (faithful to the source bug: attention is contracted with the
projected K, not V, so v/Wv are dead inputs):
    qp = q @ Wq.T ; kp = k @ Wk.T
    head split via reshape(b, l, 64, 16): head n takes strided columns {d*16+n}
    S = Qh @ Kh.T / 8 ; A = softmax(S, axis=m) ; X = A @ Kh ; out = X @ Wo.T

Strategy:
  - Host-side: permute weight rows/cols head-major so each head is a contiguous
    64-column block; pre-transpose q/k/weights into the layouts the TensorE
    wants (contraction on partitions).
  - 8 cores = 2 batches x 4 head-groups (4 heads each).  Each core computes its
    4 heads' attention plus a partial output projection; the host sums the 4
    partials per batch (tensor-parallel row-split reduction).
  - The kernel is a race between ScalarE exp (~145us of ACTIVATEs, the only
    exp engine) and TensorE (~170us of matmuls).  Everything else hides:
      * each input tile is DMA-loaded once and projected for BOTH head groups
      * attention starts right after k-tile 0 + q-tile 0 are projected; the
        remaining projections are need-ordered fillers paced through
        attention, out-projection strips pace through attention g1
      * Kh transposes (khT -> khp) run on the DMA XBAR (dma_start_transpose),
        not the PE
      * S-matmul head pairs run concurrently in the PE array (row groups)
      * X matmuls carry a fused ones-column so softmax denominators fall out
        of the X^T accumulation for free; the reciprocal+broadcast round-trip
        is deferred one strip so its DMA latency never blocks any queue
"""

import contextlib
import ctypes
import os
import sys
import types

import numpy as np

import concourse.bacc as bacc
import concourse.tile as tile
from concourse import mybir
from concourse.bass import ds, ts
from concourse.bass_utils import run_bass_kernel_spmd


def _install_ntff_hook():
    """Provide antenv.axon_hooks if the image lacks it, wiring NTFF
    profiling straight into libaxon_pjrt.so (same ABI trn_boot uses)."""
    try:
        import antenv.axon_hooks  # noqa: F401
        return
    except ImportError:
        pass
    mod = types.ModuleType("antenv.axon_hooks")
    holder = [None]
    mod.set_axon_ntff_profile_hook = lambda h: holder.__setitem__(0, h)
    mod.get_axon_ntff_profile_hook = lambda: holder[0]
    sys.modules["antenv.axon_hooks"] = mod
    try:
        import antenv
        antenv.axon_hooks = mod
    except ImportError:
        pass

    so_path = "/opt/axon/libaxon_pjrt.so"
    if not os.path.exists(so_path):
        return
    lib = ctypes.CDLL(so_path)
    if not hasattr(lib, "axon_start_nrt_profile"):
        return
    lib.axon_start_nrt_profile.argtypes = [ctypes.POINTER(ctypes.c_int64), ctypes.c_size_t]
    lib.axon_start_nrt_profile.restype = ctypes.c_int64
    lib.axon_stop_nrt_profile.argtypes = [ctypes.c_char_p]
    lib.axon_stop_nrt_profile.restype = ctypes.c_int64

    @contextlib.contextmanager
    def _hook(output_dir, device_ids):
        import jax
        jax.devices()
        if device_ids:
            ids = (ctypes.c_int64 * len(device_ids))(*device_ids)
            rc = lib.axon_start_nrt_profile(ids, len(device_ids))
        else:
            rc = lib.axon_start_nrt_profile(None, 0)
        if rc != 0:
            raise RuntimeError(f"axon_start_nrt_profile rc={rc}")
        try:
            yield
        finally:
            n = lib.axon_stop_nrt_profile(str(output_dir).encode())
            print(f"profile: {n} file(s) written to {output_dir}", file=sys.stderr)

    mod.set_axon_ntff_profile_hook(_hook)


_install_ntff_hook()

f32 = mybir.dt.float32
bf16 = mybir.dt.bfloat16
Exp = mybir.ActivationFunctionType.Exp

P = 128
DIM = 1024
NH = 16
HD = 64
HPC = 4          # heads per core
CW = HPC * HD    # 256 channel columns per core
CH = HD + 1      # head channels + ones column
G = CW // P      # 2 channel groups of 128
KC = DIM // P    # 8 contraction chunks for projections
JT = DIM // 512  # out-projection j tiles

_cache = {}


def _build(L, M):
    NT = 512                  # matmul moving-dim tile / l-strip size
    LT = L // NT              # 4 q tiles
    MT = M // NT              # 4 k tiles
    MG = M // P               # 16 m chunks for attention
    L5 = L // NT              # 4 attention l-strips per group
    LC = L // P               # 16 out-projection row chunks

    nc = bacc.Bacc()
    qT = nc.declare_dram_parameter("qT", [DIM, L], bf16, isOutput=False)
    kT = nc.declare_dram_parameter("kT", [DIM, M], bf16, isOutput=False)
    wqT = nc.declare_dram_parameter("wqT", [DIM, CW], bf16, isOutput=False)
    wkT = nc.declare_dram_parameter("wkT", [DIM, CW], bf16, isOutput=False)
    woT = nc.declare_dram_parameter("woT", [CW, DIM], bf16, isOutput=False)
    out = nc.declare_dram_parameter("out", [L, DIM], f32, isOutput=True)

    with tile.TileContext(nc) as tc:
        with (
            tc.tile_pool(name="singles", bufs=1) as singles,
            tc.tile_pool(name="io", bufs=8) as io,
            tc.tile_pool(name="es", bufs=4) as es_pool,
            tc.tile_pool(name="opool", bufs=4) as opool,
            tc.tile_pool(name="dstp", bufs=4) as dstp,
            tc.tile_pool(name="pbp", bufs=8) as pbp,
        ):
            # exp table preload: a tiny dummy activation at the head of the
            # Scalar queue pulls the ~2.7us ACT_TABLE_LOAD into the prologue
            dum = singles.tile([1, 16], f32)
            dum2 = singles.tile([1, 16], f32)
            nc.vector.memset(dum, 0.0)
            nc.scalar.activation(dum2, dum, Exp, scale=1.0)

            wk_sb = singles.tile([P, KC, CW], bf16)
            nc.sync.dma_start(wk_sb, wkT.rearrange("(kc p) c -> p kc c", p=P))
            wq_sb = singles.tile([P, KC, CW], bf16)
            wo_sb = singles.tile([P, G, DIM], bf16)

            qhT = singles.tile([P, G, L], bf16)
            khT = singles.tile([P, G, M], bf16)
            # khp: per (m-chunk, group) the two heads' Kh blocks, each with a
            # trailing ones column: [hA(64) | 1s | hB(64) | 1s]
            khp = singles.tile([P, MG, G, 2 * CH], bf16)
            xu = singles.tile([P, G, L], bf16)

            ones_sb = singles.tile([P, 1], f32)
            nc.vector.memset(ones_sb, 1.0)
            for mg in range(MG):
                for hh in range(2):
                    nc.vector.tensor_copy(
                        khp[:, mg, :, hh * CH + HD:hh * CH + CH],
                        ones_sb[:, None, :].to_broadcast([P, G, 1]))

            junkW = singles.tile([P, P], bf16)
            junkR = singles.tile([P, NT], bf16)
            nc.vector.memset(junkW, 0.0)
            nc.vector.memset(junkR, 0.0)

            with (
                tc.tile_pool(name="psP", bufs=2, space="PSUM") as psP,
                tc.tile_pool(name="psS", bufs=2, space="PSUM") as psS,
                tc.tile_pool(name="psX", bufs=2, space="PSUM") as psX,
            ):
                # warm the PE clock (HAM un-throttle needs ~3.4us of activity)
                # while the first input DMAs are still in flight
                for _ in range(18):
                    jp = psP.tile([P, NT], f32, tag="ps")
                    nc.tensor.matmul(jp, lhsT=junkW, rhs=junkR,
                                     start=True, stop=True)
                # ---- projection building blocks (one input load serves both
                # head groups) ----
                def load_tile(src_ap, tt):
                    in_t = io.tile([P, KC, NT], bf16, tag="io")
                    nc.sync.dma_start(
                        in_t, src_ap[:, ts(tt, NT)].rearrange("(kc p) l -> p kc l", p=P))
                    return in_t

                def proj_half(dst, w_sb, in_t, tt, g, half, st):
                    if half == 0:
                        ps = psP.tile([P, NT], f32, tag="ps")
                        st["ps"] = ps
                        for kc in range(KC // 2):
                            nc.tensor.matmul(ps, lhsT=w_sb[:, kc, ts(g, P)],
                                             rhs=in_t[:, kc],
                                             start=(kc == 0), stop=False)
                    else:
                        ps = st["ps"]
                        for kc in range(KC // 2, KC):
                            nc.tensor.matmul(ps, lhsT=w_sb[:, kc, ts(g, P)],
                                             rhs=in_t[:, kc],
                                             start=False, stop=(kc == KC - 1))
                        nc.vector.tensor_copy(dst[:, g, ts(tt, NT)], ps)

                from concourse.masks import make_identity
                ident = singles.tile([P, P], bf16)
                make_identity(nc, ident)

                def ktrans(mt, g):
                    # khT [c, m] -> khp [m, c], one k-tile (4 m-chunks) per call
                    for mc in range(4 * mt, 4 * mt + 4):
                        tr = psP.tile([P, P], bf16, tag="ps")
                        nc.tensor.transpose(tr, khT[:, g, ts(mc, P)], ident)
                        for hh in range(2):
                            nc.vector.tensor_copy(
                                khp[:, mc, g, hh * CH:hh * CH + HD],
                                tr[:, ts(hh, HD)])

                # ---- out-projection strip (row chunk lc, column tile jt) ----
                def outproj(lc, jt):
                    po = psP.tile([P, NT], f32, tag="ps")
                    for cc in range(G):
                        nc.tensor.matmul(po, lhsT=xu[:, cc, ts(lc, P)],
                                         rhs=wo_sb[:, cc, ts(jt, NT)],
                                         start=(cc == 0), stop=(cc == G - 1))
                    ot = opool.tile([P, NT], f32, tag="ot")
                    nc.vector.tensor_copy(ot, po)
                    nc.sync.dma_start(out[ts(lc, P), ts(jt, NT)], ot)

                # ---- DMAs: k-side on the sync queue, q-side on the scalar
                # queue so the two streams load concurrently and the prologue
                # projections start ~4us in
                def load_tile_q(src_ap, tt):
                    in_t = io.tile([P, KC, NT], bf16, tag="io")
                    nc.scalar.dma_start(
                        in_t, src_ap[:, ts(tt, NT)].rearrange("(kc p) l -> p kc l", p=P))
                    return in_t

                kt0 = load_tile(kT, 0)
                nc.scalar.dma_start(wq_sb, wqT.rearrange("(kc p) c -> p kc c", p=P))
                qt0 = load_tile_q(qT, 0)
                kt1 = load_tile(kT, 1)
                kt2 = load_tile(kT, 2)
                kt3 = load_tile(kT, 3)
                qt1 = load_tile_q(qT, 1)
                qt2 = load_tile_q(qT, 2)
                qt3 = load_tile_q(qT, 3)
                nc.scalar.dma_start(wo_sb, woT.rearrange("(g p) j -> p g j", p=P))

                # ---- prologue: the minimum compute before attention starts
                st = {}
                proj_half(khT, wk_sb, kt0, 0, 0, 0, st)
                proj_half(khT, wk_sb, kt0, 0, 0, 1, st)
                ktrans(0, 0)
                st = {}
                proj_half(qhT, wq_sb, qt0, 0, 0, 0, st)
                proj_half(qhT, wq_sb, qt0, 0, 0, 1, st)

                # ---- filler atoms, ordered by first use ----
                fillers = []

                def add_proj_atoms(dst, w_sb, in_t, tt, g):
                    st = {}
                    fillers.append(lambda st=st: proj_half(dst, w_sb, in_t, tt, g, 0, st))
                    fillers.append(lambda st=st: proj_half(dst, w_sb, in_t, tt, g, 1, st))

                add_proj_atoms(khT, wk_sb, kt1, 1, 0)
                fillers.append(lambda: ktrans(1, 0))
                add_proj_atoms(khT, wk_sb, kt2, 2, 0)
                fillers.append(lambda: ktrans(2, 0))
                add_proj_atoms(khT, wk_sb, kt3, 3, 0)
                fillers.append(lambda: ktrans(3, 0))
                add_proj_atoms(qhT, wq_sb, qt1, 1, 0)
                add_proj_atoms(khT, wk_sb, kt0, 0, 1)
                fillers.append(lambda: ktrans(0, 1))
                add_proj_atoms(khT, wk_sb, kt1, 1, 1)
                fillers.append(lambda: ktrans(1, 1))
                add_proj_atoms(khT, wk_sb, kt2, 2, 1)
                fillers.append(lambda: ktrans(2, 1))
                add_proj_atoms(khT, wk_sb, kt3, 3, 1)
                fillers.append(lambda: ktrans(3, 1))
                add_proj_atoms(qhT, wq_sb, qt2, 2, 0)
                add_proj_atoms(qhT, wq_sb, qt3, 3, 0)
                add_proj_atoms(qhT, wq_sb, qt0, 0, 1)
                add_proj_atoms(qhT, wq_sb, qt1, 1, 1)
                add_proj_atoms(qhT, wq_sb, qt2, 2, 1)
                add_proj_atoms(qhT, wq_sb, qt3, 3, 1)

                # ---- attention ----
                def emit_sp(g, lsl, mc):
                    sps = psS.tile([P, 2 * NT], f32, tag="s")
                    nc.tensor.matmul(sps[:, 0:NT],
                                     lhsT=khT[0:HD, g, ts(mc, P)],
                                     rhs=qhT[0:HD, g, lsl],
                                     start=True, stop=True)
                    nc.tensor.matmul(sps[:, NT:2 * NT],
                                     lhsT=khT[HD:P, g, ts(mc, P)],
                                     rhs=qhT[HD:P, g, lsl],
                                     start=True, stop=True)
                    return sps

                def normalize_atoms(g, lsl, dbcs):
                    # deferred one strip: the partition_broadcast issued at the
                    # end of strip (g,l5) has long completed, so the reciprocal
                    # never waits inside the Vector queue; four small atoms so
                    # the Vector queue never gets a long normalize block
                    atoms = []
                    rts = [None, None]
                    for hh in range(2):
                        def recip(hh=hh):
                            # NB: reciprocal_approx_fast mishandles APs with a
                            # non-zero base partition - always full-tile it
                            rt = pbp.tile([P, NT], f32, tag="pb")
                            rts[hh] = rt
                            nc.vector.reciprocal_approx_fast(
                                out=rt, in_=dbcs[hh])
                        def mul(hh=hh):
                            nc.vector.tensor_mul(xu[ts(hh, HD), g, lsl],
                                                 xu[ts(hh, HD), g, lsl],
                                                 rts[hh][ts(hh, HD)])
                        atoms.append(recip)
                        atoms.append(mul)
                    return atoms

                for g in range(G):
                    for l5 in range(L5):
                        lsl = ts(l5, NT)

                        xpsA = psX.tile([CH, NT], f32, tag="x")
                        xpsB = psX.tile([CH, NT], f32, tag="x")
                        sq = [emit_sp(g, lsl, 0)]
                        if MG > 1:
                            sq.append(emit_sp(g, lsl, 1))
                        for mc in range(MG):
                            if mc + 2 < MG:
                                sq.append(emit_sp(g, lsl, mc + 2))
                            es = es_pool.tile([P, 2 * NT], bf16, tag="es")
                            nc.scalar.activation(es, sq.pop(0), Exp, scale=0.125)
                            if fillers:
                                fillers.pop(0)()
                            nc.tensor.matmul(xpsA, lhsT=khp[:, mc, g, 0:CH],
                                             rhs=es[:, 0:NT],
                                             start=(mc == 0), stop=(mc == MG - 1))
                            nc.tensor.matmul(xpsB, lhsT=khp[:, mc, g, CH:2 * CH],
                                             rhs=es[:, NT:2 * NT],
                                             start=(mc == 0), stop=(mc == MG - 1))

                        dbcs = []
                        for hh, xps in ((0, xpsA), (1, xpsB)):
                            nc.vector.tensor_copy(xu[ts(hh, HD), g, lsl], xps[0:HD])
                            dstg = dstp.tile([1, NT], f32, tag="dst")
                            nc.vector.tensor_copy(dstg, xps[HD:CH])
                            dbc = pbp.tile([P, NT], f32, tag="pb")
                            nc.gpsimd.partition_broadcast(dbc, dstg)
                            dbcs.append(dbc)
                        # reciprocal + multiply deferred one strip
                        for i, atom in enumerate(normalize_atoms(g, lsl, dbcs)):
                            fillers.insert(i, atom)

                        if g == 1:
                            # out-projection for this strip becomes filler work
                            # paced through the next strip; it sits after this
                            # strip's normalize atom, so it reads normalized xu
                            for lc in range(l5 * (NT // P), (l5 + 1) * (NT // P)):
                                for jt in range(JT):
                                    fillers.append(
                                        lambda lc=lc, jt=jt: outproj(lc, jt))

                while fillers:
                    fillers.pop(0)()

    nc.finalize()
    return nc


def _get_nc(L, M):
    key = (L, M)
    if key not in _cache:
        _cache[key] = _build(L, M)
    return _cache[key]


# head-major channel permutation: new channel c = h*64+d <- original column d*16+h
_PERM = np.array([(c % HD) * NH + c // HD for c in range(DIM)])

last_exec_time_ns = None
last_results = None


def kernel(q, k, v, Wq, Wk, Wv, Wo):  # noqa: ARG001 - v/Wv dead in reference
    global last_exec_time_ns, last_results
    q = np.asarray(q, np.float32)
    k = np.asarray(k, np.float32)
    Wq = np.asarray(Wq, np.float32)
    Wk = np.asarray(Wk, np.float32)
    Wo = np.asarray(Wo, np.float32)
    B, L, _ = q.shape
    M = k.shape[1]

    import ml_dtypes
    bf = ml_dtypes.bfloat16
    Wq_p = Wq[_PERM]            # (1024, 1024) head-major rows
    Wk_p = Wk[_PERM]
    WoT_p = Wo[:, _PERM].T      # (1024 c, 1024 j)

    qT = [np.ascontiguousarray(q[b].T).astype(bf) for b in range(B)]
    kT = [np.ascontiguousarray(k[b].T).astype(bf) for b in range(B)]
    wqT = [np.ascontiguousarray(Wq_p[hg * CW:(hg + 1) * CW, :].T).astype(bf) for hg in range(4)]
    wkT = [np.ascontiguousarray(Wk_p[hg * CW:(hg + 1) * CW, :].T).astype(bf) for hg in range(4)]
    woT = [np.ascontiguousarray(WoT_p[hg * CW:(hg + 1) * CW, :]).astype(bf) for hg in range(4)]

    in_maps = []
    for core in range(8):
        b, hg = divmod(core, 4)
        in_maps.append({"qT": qT[b], "kT": kT[b], "wqT": wqT[hg],
                        "wkT": wkT[hg], "woT": woT[hg]})

    nc = _get_nc(L, M)
    trace = bool(int(os.environ.get("MHA_TRACE", "0")))
    res = run_bass_kernel_spmd(nc, in_maps, core_ids=list(range(8)), trace=trace)
    last_results = res
    last_exec_time_ns = res.exec_time_ns

    out = np.zeros((B, L, DIM), np.float32)
    for core in range(8):
        b = core // 4
        out[b] += res.results[core]["out"]
    return out


# revision 33
# speedup vs baseline: 1.2016x; 1.0258x over previous
"""Trainium2 Bass kernel for nn_MultiHeadAttention_60559038873660.

Reference math (faithful to the source bug: attention is contracted with the
projected K, not V, so v/Wv are dead inputs):
    qp = q @ Wq.T ; kp = k @ Wk.T
    head split via reshape(b, l, 64, 16): head n takes strided columns {d*16+n}
    S = Qh @ Kh.T / 8 ; A = softmax(S, axis=m) ; X = A @ Kh ; out = X @ Wo.T

Strategy:
  - Host-side: permute weight rows/cols head-major so each head is a contiguous
    64-column block; pre-transpose q/k/weights into the layouts the TensorE
    wants (contraction on partitions).
  - 8 cores = 2 batches x 4 head-groups (4 heads each).  Each core computes its
    4 heads' attention plus a partial output projection; the host sums the 4
    partials per batch (tensor-parallel row-split reduction).
  - The kernel is a race between ScalarE exp (~145us of ACTIVATEs, the only
    exp engine) and TensorE (~170us of matmuls).  Everything else hides:
      * each input tile is DMA-loaded once and projected for BOTH head groups
      * attention starts right after k-tile 0 + q-tile 0 are projected; the
        remaining projections are need-ordered fillers paced through
        attention, out-projection strips pace through attention g1
      * Kh transposes (khT -> khp) run on the DMA XBAR (dma_start_transpose),
        not the PE
      * S-matmul head pairs run concurrently in the PE array (row groups)
      * X matmuls carry a fused ones-column so softmax denominators fall out
        of the X^T accumulation for free; the reciprocal+broadcast round-trip
        is deferred one strip so its DMA latency never blocks any queue
"""

import contextlib
import ctypes
import os
import sys
import types

import numpy as np

import concourse.bacc as bacc
import concourse.tile as tile
from concourse import mybir
from concourse.bass import ds, ts
from concourse.bass_utils import run_bass_kernel_spmd


def _install_ntff_hook():
    """Provide antenv.axon_hooks if the image lacks it, wiring NTFF
    profiling straight into libaxon_pjrt.so (same ABI trn_boot uses)."""
    try:
        import antenv.axon_hooks  # noqa: F401
        return
    except ImportError:
        pass
    mod = types.ModuleType("antenv.axon_hooks")
    holder = [None]
    mod.set_axon_ntff_profile_hook = lambda h: holder.__setitem__(0, h)
    mod.get_axon_ntff_profile_hook = lambda: holder[0]
    sys.modules["antenv.axon_hooks"] = mod
    try:
        import antenv
        antenv.axon_hooks = mod
    except ImportError:
        pass

    so_path = "/opt/axon/libaxon_pjrt.so"
    if not os.path.exists(so_path):
        return
    lib = ctypes.CDLL(so_path)
    if not hasattr(lib, "axon_start_nrt_profile"):
        return
    lib.axon_start_nrt_profile.argtypes = [ctypes.POINTER(ctypes.c_int64), ctypes.c_size_t]
    lib.axon_start_nrt_profile.restype = ctypes.c_int64
    lib.axon_stop_nrt_profile.argtypes = [ctypes.c_char_p]
    lib.axon_stop_nrt_profile.restype = ctypes.c_int64

    @contextlib.contextmanager
    def _hook(output_dir, device_ids):
        import jax
        jax.devices()
        if device_ids:
            ids = (ctypes.c_int64 * len(device_ids))(*device_ids)
            rc = lib.axon_start_nrt_profile(ids, len(device_ids))
        else:
            rc = lib.axon_start_nrt_profile(None, 0)
        if rc != 0:
            raise RuntimeError(f"axon_start_nrt_profile rc={rc}")
        try:
            yield
        finally:
            n = lib.axon_stop_nrt_profile(str(output_dir).encode())
            print(f"profile: {n} file(s) written to {output_dir}", file=sys.stderr)

    mod.set_axon_ntff_profile_hook(_hook)


_install_ntff_hook()

f32 = mybir.dt.float32
bf16 = mybir.dt.bfloat16
Exp = mybir.ActivationFunctionType.Exp

P = 128
DIM = 1024
NH = 16
HD = 64
HPC = 4          # heads per core
CW = HPC * HD    # 256 channel columns per core
CH = HD + 1      # head channels + ones column
G = CW // P      # 2 channel groups of 128
KC = DIM // P    # 8 contraction chunks for projections
JT = DIM // 512  # out-projection j tiles

_cache = {}


def _build(L, M):
    NT = 512                  # matmul moving-dim tile / l-strip size
    LT = L // NT              # 4 q tiles
    MT = M // NT              # 4 k tiles
    MG = M // P               # 16 m chunks for attention
    L5 = L // NT              # 4 attention l-strips per group
    LC = L // P               # 16 out-projection row chunks

    nc = bacc.Bacc()
    # all inputs arrive in on-chip-tile-contiguous layouts (host pre-arranged)
    # so every load is a full-rate contiguous DMA
    qT = nc.declare_dram_parameter("qT", [L // NT, P, KC, NT], bf16, isOutput=False)
    kT = nc.declare_dram_parameter("kT", [M // NT, P, KC, NT], bf16, isOutput=False)
    wqT = nc.declare_dram_parameter("wqT", [P, KC, CW], bf16, isOutput=False)
    wkT = nc.declare_dram_parameter("wkT", [P, KC, CW], bf16, isOutput=False)
    woT = nc.declare_dram_parameter("woT", [P, G, DIM], bf16, isOutput=False)
    out = nc.declare_dram_parameter("out", [L, DIM], f32, isOutput=True)

    with tile.TileContext(nc) as tc:
        with (
            tc.tile_pool(name="singles", bufs=1) as singles,
            tc.tile_pool(name="io", bufs=8) as io,
            tc.tile_pool(name="es", bufs=4) as es_pool,
            tc.tile_pool(name="opool", bufs=4) as opool,
            tc.tile_pool(name="dstp", bufs=4) as dstp,
            tc.tile_pool(name="pbp", bufs=8) as pbp,
        ):
            # exp table preload: a tiny dummy activation at the head of the
            # Scalar queue pulls the ~2.7us ACT_TABLE_LOAD into the prologue
            dum = singles.tile([1, 16], f32)
            dum2 = singles.tile([1, 16], f32)
            nc.vector.memset(dum, 0.0)
            nc.scalar.activation(dum2, dum, Exp, scale=1.0)

            wk_sb = singles.tile([P, KC, CW], bf16)
            nc.sync.dma_start(wk_sb, wkT[:, :])
            wq_sb = singles.tile([P, KC, CW], bf16)
            wo_sb = singles.tile([P, G, DIM], bf16)

            qhT = singles.tile([P, G, L], bf16)
            khT = singles.tile([P, G, M], bf16)
            # khp: per (m-chunk, group) the two heads' Kh blocks, each with a
            # trailing ones column: [hA(64) | 1s | hB(64) | 1s]
            khp = singles.tile([P, MG, G, 2 * CH], bf16)
            xu = singles.tile([P, G, L], bf16)

            ones_sb = singles.tile([P, 1], f32)
            nc.vector.memset(ones_sb, 1.0)
            for mg in range(MG):
                for hh in range(2):
                    nc.vector.tensor_copy(
                        khp[:, mg, :, hh * CH + HD:hh * CH + CH],
                        ones_sb[:, None, :].to_broadcast([P, G, 1]))

            junkW = singles.tile([P, P], bf16)
            junkR = singles.tile([P, NT], bf16)
            nc.vector.memset(junkW, 0.0)
            nc.vector.memset(junkR, 0.0)

            with (
                tc.tile_pool(name="psP", bufs=2, space="PSUM") as psP,
                tc.tile_pool(name="psS", bufs=2, space="PSUM") as psS,
                tc.tile_pool(name="psX", bufs=2, space="PSUM") as psX,
            ):
                # warm the PE clock (HAM un-throttle needs ~3.4us of activity)
                # while the first input DMAs are still in flight
                for _ in range(18):
                    jp = psP.tile([P, NT], f32, tag="ps")
                    nc.tensor.matmul(jp, lhsT=junkW, rhs=junkR,
                                     start=True, stop=True)
                # ---- projection building blocks (one input load serves both
                # head groups) ----
                def load_tile(src_ap, tt):
                    in_t = io.tile([P, KC, NT], bf16, tag="io")
                    nc.sync.dma_start(in_t, src_ap[tt])
                    return in_t

                def proj_half(dst, w_sb, in_t, tt, g, half, st):
                    if half == 0:
                        ps = psP.tile([P, NT], f32, tag="ps")
                        st["ps"] = ps
                        for kc in range(KC // 2):
                            nc.tensor.matmul(ps, lhsT=w_sb[:, kc, ts(g, P)],
                                             rhs=in_t[:, kc],
                                             start=(kc == 0), stop=False)
                    else:
                        ps = st["ps"]
                        for kc in range(KC // 2, KC):
                            nc.tensor.matmul(ps, lhsT=w_sb[:, kc, ts(g, P)],
                                             rhs=in_t[:, kc],
                                             start=False, stop=(kc == KC - 1))
                        nc.vector.tensor_copy(dst[:, g, ts(tt, NT)], ps)

                from concourse.masks import make_identity
                ident = singles.tile([P, P], bf16)
                make_identity(nc, ident)

                def ktrans(mt, g):
                    # khT [c, m] -> khp [m, c], one k-tile (4 m-chunks) per call
                    for mc in range(4 * mt, 4 * mt + 4):
                        tr = psP.tile([P, P], bf16, tag="ps")
                        nc.tensor.transpose(tr, khT[:, g, ts(mc, P)], ident)
                        for hh in range(2):
                            nc.vector.tensor_copy(
                                khp[:, mc, g, hh * CH:hh * CH + HD],
                                tr[:, ts(hh, HD)])

                # ---- out-projection strip (row chunk lc, column tile jt) ----
                def outproj(lc, jt):
                    po = psP.tile([P, NT], f32, tag="ps")
                    for cc in range(G):
                        nc.tensor.matmul(po, lhsT=xu[:, cc, ts(lc, P)],
                                         rhs=wo_sb[:, cc, ts(jt, NT)],
                                         start=(cc == 0), stop=(cc == G - 1))
                    ot = opool.tile([P, NT], f32, tag="ot")
                    nc.vector.tensor_copy(ot, po)
                    nc.sync.dma_start(out[ts(lc, P), ts(jt, NT)], ot)

                # ---- DMAs: k-side on the sync queue, q-side on the scalar
                # queue so the two streams load concurrently and the prologue
                # projections start ~4us in
                def load_tile_q(src_ap, tt):
                    in_t = io.tile([P, KC, NT], bf16, tag="io")
                    nc.scalar.dma_start(in_t, src_ap[tt])
                    return in_t

                kt0 = load_tile(kT, 0)
                nc.scalar.dma_start(wq_sb, wqT[:, :])
                qt0 = load_tile_q(qT, 0)
                kt1 = load_tile(kT, 1)
                kt2 = load_tile(kT, 2)
                kt3 = load_tile(kT, 3)
                qt1 = load_tile_q(qT, 1)
                qt2 = load_tile_q(qT, 2)
                qt3 = load_tile_q(qT, 3)
                nc.scalar.dma_start(wo_sb, woT[:, :])

                # ---- prologue: the minimum compute before attention starts
                st = {}
                proj_half(khT, wk_sb, kt0, 0, 0, 0, st)
                proj_half(khT, wk_sb, kt0, 0, 0, 1, st)
                ktrans(0, 0)
                st = {}
                proj_half(qhT, wq_sb, qt0, 0, 0, 0, st)
                proj_half(qhT, wq_sb, qt0, 0, 0, 1, st)

                # ---- filler atoms, ordered by first use ----
                fillers = []

                def add_proj_atoms(dst, w_sb, in_t, tt, g):
                    st = {}
                    fillers.append(lambda st=st: proj_half(dst, w_sb, in_t, tt, g, 0, st))
                    fillers.append(lambda st=st: proj_half(dst, w_sb, in_t, tt, g, 1, st))

                add_proj_atoms(khT, wk_sb, kt1, 1, 0)
                fillers.append(lambda: ktrans(1, 0))
                add_proj_atoms(khT, wk_sb, kt2, 2, 0)
                fillers.append(lambda: ktrans(2, 0))
                add_proj_atoms(khT, wk_sb, kt3, 3, 0)
                fillers.append(lambda: ktrans(3, 0))
                add_proj_atoms(qhT, wq_sb, qt1, 1, 0)
                add_proj_atoms(khT, wk_sb, kt0, 0, 1)
                fillers.append(lambda: ktrans(0, 1))
                add_proj_atoms(khT, wk_sb, kt1, 1, 1)
                fillers.append(lambda: ktrans(1, 1))
                add_proj_atoms(khT, wk_sb, kt2, 2, 1)
                fillers.append(lambda: ktrans(2, 1))
                add_proj_atoms(khT, wk_sb, kt3, 3, 1)
                fillers.append(lambda: ktrans(3, 1))
                add_proj_atoms(qhT, wq_sb, qt2, 2, 0)
                add_proj_atoms(qhT, wq_sb, qt3, 3, 0)
                add_proj_atoms(qhT, wq_sb, qt0, 0, 1)
                add_proj_atoms(qhT, wq_sb, qt1, 1, 1)
                add_proj_atoms(qhT, wq_sb, qt2, 2, 1)
                add_proj_atoms(qhT, wq_sb, qt3, 3, 1)

                # ---- attention ----
                def emit_sp(g, lsl, mc):
                    sps = psS.tile([P, 2 * NT], f32, tag="s")
                    with tc.high_priority(offset=32):
                        nc.tensor.matmul(sps[:, 0:NT],
                                         lhsT=khT[0:HD, g, ts(mc, P)],
                                         rhs=qhT[0:HD, g, lsl],
                                         start=True, stop=True)
                        nc.tensor.matmul(sps[:, NT:2 * NT],
                                         lhsT=khT[HD:P, g, ts(mc, P)],
                                         rhs=qhT[HD:P, g, lsl],
                                         start=True, stop=True)
                    return sps

                def normalize_atoms(g, lsl, dbcs):
                    # deferred one strip: the partition_broadcast issued at the
                    # end of strip (g,l5) has long completed, so the reciprocal
                    # never waits inside the Vector queue; four small atoms so
                    # the Vector queue never gets a long normalize block
                    atoms = []
                    rts = [None, None]
                    for hh in range(2):
                        def recip(hh=hh):
                            # NB: reciprocal_approx_fast mishandles APs with a
                            # non-zero base partition - always full-tile it
                            rt = pbp.tile([P, NT], f32, tag="pb")
                            rts[hh] = rt
                            nc.vector.reciprocal_approx_fast(
                                out=rt, in_=dbcs[hh])
                        def mul(hh=hh):
                            nc.vector.tensor_mul(xu[ts(hh, HD), g, lsl],
                                                 xu[ts(hh, HD), g, lsl],
                                                 rts[hh][ts(hh, HD)])
                        atoms.append((recip, mul))
                    # recips first (independent), then the multiplies
                    return [atoms[0][0], atoms[1][0], atoms[0][1], atoms[1][1]]

                for g in range(G):
                    for l5 in range(L5):
                        lsl = ts(l5, NT)

                        xpsA = psX.tile([CH, NT], f32, tag="x")
                        xpsB = psX.tile([CH, NT], f32, tag="x")
                        sq = [emit_sp(g, lsl, 0)]
                        if MG > 1:
                            sq.append(emit_sp(g, lsl, 1))
                        for mc in range(MG):
                            if mc + 2 < MG:
                                sq.append(emit_sp(g, lsl, mc + 2))
                            es = es_pool.tile([P, 2 * NT], bf16, tag="es")
                            nc.scalar.activation(es, sq.pop(0), Exp, scale=0.125)
                            if fillers:
                                fillers.pop(0)()
                            nc.tensor.matmul(xpsA, lhsT=khp[:, mc, g, 0:CH],
                                             rhs=es[:, 0:NT],
                                             start=(mc == 0), stop=(mc == MG - 1))
                            nc.tensor.matmul(xpsB, lhsT=khp[:, mc, g, CH:2 * CH],
                                             rhs=es[:, NT:2 * NT],
                                             start=(mc == 0), stop=(mc == MG - 1))

                        dbcs = []
                        for hh, xps in ((0, xpsA), (1, xpsB)):
                            nc.vector.tensor_copy(xu[ts(hh, HD), g, lsl], xps[0:HD])
                            dstg = dstp.tile([1, NT], f32, tag="dst")
                            nc.vector.tensor_copy(dstg, xps[HD:CH])
                            dbc = pbp.tile([P, NT], f32, tag="pb")
                            nc.gpsimd.partition_broadcast(dbc, dstg)
                            dbcs.append(dbc)
                        # reciprocal + multiply deferred one strip
                        for i, atom in enumerate(normalize_atoms(g, lsl, dbcs)):
                            fillers.insert(i, atom)

                        if g == 1:
                            # out-projection for this strip becomes filler work
                            # paced through the next strip; it sits after this
                            # strip's normalize atom, so it reads normalized xu
                            for lc in range(l5 * (NT // P), (l5 + 1) * (NT // P)):
                                for jt in range(JT):
                                    fillers.append(
                                        lambda lc=lc, jt=jt: outproj(lc, jt))

                while fillers:
                    fillers.pop(0)()

    nc.finalize()
    return nc


def _get_nc(L, M):
    key = (L, M)
    if key not in _cache:
        _cache[key] = _build(L, M)
    return _cache[key]


# head-major channel permutation: new channel c = h*64+d <- original column d*16+h
_PERM = np.array([(c % HD) * NH + c // HD for c in range(DIM)])

last_exec_time_ns = None
last_results = None


def kernel(q, k, v, Wq, Wk, Wv, Wo):  # noqa: ARG001 - v/Wv dead in reference
    global last_exec_time_ns, last_results
    q = np.asarray(q, np.float32)
    k = np.asarray(k, np.float32)
    Wq = np.asarray(Wq, np.float32)
    Wk = np.asarray(Wk, np.float32)
    Wo = np.asarray(Wo, np.float32)
    B, L, _ = q.shape
    M = k.shape[1]

    import ml_dtypes
    bf = ml_dtypes.bfloat16
    Wq_p = Wq[_PERM]            # (1024, 1024) head-major rows
    Wk_p = Wk[_PERM]
    WoT_p = Wo[:, _PERM].T      # (1024 c, 1024 j)

    NT, LT, MT = 512, L // 512, M // 512

    def tiles4(xT):             # [DIM, L] -> [LT, P, KC, NT] tile-contiguous
        return np.ascontiguousarray(
            xT.reshape(KC, P, LT, NT).transpose(2, 1, 0, 3)).astype(bf)

    def wprep(w):               # [DIM, CW] -> [P, KC, CW]
        return np.ascontiguousarray(
            w.reshape(KC, P, CW).transpose(1, 0, 2)).astype(bf)

    qT = [tiles4(q[b].T) for b in range(B)]
    kT = [tiles4(k[b].T) for b in range(B)]
    wqT = [wprep(Wq_p[hg * CW:(hg + 1) * CW, :].T) for hg in range(4)]
    wkT = [wprep(Wk_p[hg * CW:(hg + 1) * CW, :].T) for hg in range(4)]
    woT = [np.ascontiguousarray(
        WoT_p[hg * CW:(hg + 1) * CW, :].reshape(2, P, DIM).transpose(1, 0, 2)
    ).astype(bf) for hg in range(4)]

    in_maps = []
    for core in range(8):
        b, hg = divmod(core, 4)
        in_maps.append({"qT": qT[b], "kT": kT[b], "wqT": wqT[hg],
                        "wkT": wkT[hg], "woT": woT[hg]})

    nc = _get_nc(L, M)
    trace = bool(int(os.environ.get("MHA_TRACE", "0")))
    res = run_bass_kernel_spmd(nc, in_maps, core_ids=list(range(8)), trace=trace)
    last_results = res
    last_exec_time_ns = res.exec_time_ns

    out = np.zeros((B, L, DIM), np.float32)
    for core in range(8):
        b = core // 4
        out[b] += res.results[core]["out"]
    return out


# revision 40
# speedup vs baseline: 1.2205x; 1.0157x over previous
"""Trainium2 Bass kernel for nn_MultiHeadAttention_60559038873660.

Reference math (faithful to the source bug: attention is contracted with the
projected K, not V, so v/Wv are dead inputs):
    qp = q @ Wq.T ; kp = k @ Wk.T
    head split via reshape(b, l, 64, 16): head n takes strided columns {d*16+n}
    S = Qh @ Kh.T / 8 ; A = softmax(S, axis=m) ; X = A @ Kh ; out = X @ Wo.T

Strategy:
  - Host-side: permute weight rows/cols head-major so each head is a contiguous
    64-column block; pre-transpose q/k/weights into the layouts the TensorE
    wants (contraction on partitions).
  - 8 cores = 2 batches x 4 head-groups (4 heads each).  Each core computes its
    4 heads' attention plus a partial output projection; the host sums the 4
    partials per batch (tensor-parallel row-split reduction).
  - The kernel is a race between ScalarE exp (~145us of ACTIVATEs, the only
    exp engine) and TensorE (~170us of matmuls).  Everything else hides:
      * each input tile is DMA-loaded once and projected for BOTH head groups
      * attention starts right after k-tile 0 + q-tile 0 are projected; the
        remaining projections are need-ordered fillers paced through
        attention, out-projection strips pace through attention g1
      * Kh transposes (khT -> khp) run on the DMA XBAR (dma_start_transpose),
        not the PE
      * S-matmul head pairs run concurrently in the PE array (row groups)
      * X matmuls carry a fused ones-column so softmax denominators fall out
        of the X^T accumulation for free; the reciprocal+broadcast round-trip
        is deferred one strip so its DMA latency never blocks any queue
"""

import contextlib
import ctypes
import os
import sys
import types

import numpy as np

import concourse.bacc as bacc
import concourse.tile as tile
from concourse import mybir
from concourse.bass import ds, ts
from concourse.bass_utils import run_bass_kernel_spmd


def _install_ntff_hook():
    """Provide antenv.axon_hooks if the image lacks it, wiring NTFF
    profiling straight into libaxon_pjrt.so (same ABI trn_boot uses)."""
    try:
        import antenv.axon_hooks  # noqa: F401
        return
    except ImportError:
        pass
    mod = types.ModuleType("antenv.axon_hooks")
    holder = [None]
    mod.set_axon_ntff_profile_hook = lambda h: holder.__setitem__(0, h)
    mod.get_axon_ntff_profile_hook = lambda: holder[0]
    sys.modules["antenv.axon_hooks"] = mod
    try:
        import antenv
        antenv.axon_hooks = mod
    except ImportError:
        pass

    so_path = "/opt/axon/libaxon_pjrt.so"
    if not os.path.exists(so_path):
        return
    lib = ctypes.CDLL(so_path)
    if not hasattr(lib, "axon_start_nrt_profile"):
        return
    lib.axon_start_nrt_profile.argtypes = [ctypes.POINTER(ctypes.c_int64), ctypes.c_size_t]
    lib.axon_start_nrt_profile.restype = ctypes.c_int64
    lib.axon_stop_nrt_profile.argtypes = [ctypes.c_char_p]
    lib.axon_stop_nrt_profile.restype = ctypes.c_int64

    @contextlib.contextmanager
    def _hook(output_dir, device_ids):
        import jax
        jax.devices()
        if device_ids:
            ids = (ctypes.c_int64 * len(device_ids))(*device_ids)
            rc = lib.axon_start_nrt_profile(ids, len(device_ids))
        else:
            rc = lib.axon_start_nrt_profile(None, 0)
        if rc != 0:
            raise RuntimeError(f"axon_start_nrt_profile rc={rc}")
        try:
            yield
        finally:
            n = lib.axon_stop_nrt_profile(str(output_dir).encode())
            print(f"profile: {n} file(s) written to {output_dir}", file=sys.stderr)

    mod.set_axon_ntff_profile_hook(_hook)


_install_ntff_hook()

f32 = mybir.dt.float32
bf16 = mybir.dt.bfloat16
Exp = mybir.ActivationFunctionType.Exp

P = 128
DIM = 1024
NH = 16
HD = 64
HPC = 4          # heads per core
CW = HPC * HD    # 256 channel columns per core
CH = HD + 1      # head channels + ones column
G = CW // P      # 2 channel groups of 128
KC = DIM // P    # 8 contraction chunks for projections
JT = DIM // 512  # out-projection j tiles

_cache = {}


def _build(L, M):
    NT = 512                  # matmul moving-dim tile / l-strip size
    LT = L // NT              # 4 q tiles
    MT = M // NT              # 4 k tiles
    MG = M // P               # 16 m chunks for attention
    L5 = L // NT              # 4 attention l-strips per group
    LC = L // P               # 16 out-projection row chunks

    nc = bacc.Bacc()
    # all inputs arrive in on-chip-tile-contiguous layouts (host pre-arranged)
    # so every load is a full-rate contiguous DMA
    qT = nc.declare_dram_parameter("qT", [L // NT, P, KC, NT], bf16, isOutput=False)
    kT = nc.declare_dram_parameter("kT", [M // NT, P, KC, NT], bf16, isOutput=False)
    wqT = nc.declare_dram_parameter("wqT", [P, KC, CW], bf16, isOutput=False)
    wkT = nc.declare_dram_parameter("wkT", [P, KC, CW], bf16, isOutput=False)
    woT = nc.declare_dram_parameter("woT", [P, G, DIM], bf16, isOutput=False)
    out = nc.declare_dram_parameter("out", [L, DIM], f32, isOutput=True)

    with tile.TileContext(nc) as tc:
        with (
            tc.tile_pool(name="singles", bufs=1) as singles,
            tc.tile_pool(name="io", bufs=8) as io,
            tc.tile_pool(name="es", bufs=4) as es_pool,
            tc.tile_pool(name="opool", bufs=4) as opool,
            tc.tile_pool(name="dstp", bufs=4) as dstp,
            tc.tile_pool(name="pbp", bufs=8) as pbp,
            tc.tile_pool(name="stp", bufs=2) as stp,
        ):
            # exp table preload: a tiny dummy activation at the head of the
            # Scalar queue pulls the ~2.7us ACT_TABLE_LOAD into the prologue
            dum = singles.tile([1, 16], f32)
            dum2 = singles.tile([1, 16], f32)
            nc.vector.memset(dum, 0.0)
            nc.scalar.activation(dum2, dum, Exp, scale=1.0)

            wk_sb = singles.tile([P, KC, CW], bf16)
            nc.sync.dma_start(wk_sb, wkT[:, :])
            wq_sb = singles.tile([P, KC, CW], bf16)
            wo_sb = singles.tile([P, G, DIM], bf16)

            qhT = singles.tile([P, G, L], bf16)
            khT = singles.tile([P, G, M], bf16)
            # khp: per (m-chunk, group) the two heads' Kh blocks, each with a
            # trailing ones column: [hA(64) | 1s | hB(64) | 1s]
            khp = singles.tile([P, MG, G, 2 * CH], bf16)
            xu = singles.tile([P, G, L], bf16)

            ones_sb = singles.tile([P, 1], f32)
            nc.vector.memset(ones_sb, 1.0)
            for mg in range(MG):
                for hh in range(2):
                    nc.vector.tensor_copy(
                        khp[:, mg, :, hh * CH + HD:hh * CH + CH],
                        ones_sb[:, None, :].to_broadcast([P, G, 1]))

            junkW = singles.tile([P, P], bf16)
            junkR = singles.tile([P, NT], bf16)
            nc.vector.memset(junkW, 0.0)
            nc.vector.memset(junkR, 0.0)

            with (
                tc.tile_pool(name="psP", bufs=2, space="PSUM") as psP,
                tc.tile_pool(name="psS", bufs=2, space="PSUM") as psS,
                tc.tile_pool(name="psX", bufs=2, space="PSUM") as psX,
            ):
                # warm the PE clock (HAM un-throttle needs ~3.4us of activity)
                # while the first input DMAs are still in flight; one psum
                # accumulation group so no WAW semaphores serialize the burst
                NJ = 14
                jp = psP.tile([P, NT], f32, tag="ps")
                for i in range(NJ):
                    nc.tensor.matmul(jp, lhsT=junkW, rhs=junkR,
                                     start=(i == 0), stop=(i == NJ - 1))
                # ---- projection building blocks (one input load serves both
                # head groups) ----
                def load_tile(src_ap, tt):
                    in_t = io.tile([P, KC, NT], bf16, tag="io")
                    nc.sync.dma_start(in_t, src_ap[tt])
                    return in_t

                def proj_half(dst, w_sb, in_t, tt, g, half, st):
                    if half == 0:
                        ps = psP.tile([P, NT], f32, tag="ps")
                        st["ps"] = ps
                        for kc in range(KC // 2):
                            nc.tensor.matmul(ps, lhsT=w_sb[:, kc, ts(g, P)],
                                             rhs=in_t[:, kc],
                                             start=(kc == 0), stop=False)
                    else:
                        ps = st["ps"]
                        for kc in range(KC // 2, KC):
                            nc.tensor.matmul(ps, lhsT=w_sb[:, kc, ts(g, P)],
                                             rhs=in_t[:, kc],
                                             start=False, stop=(kc == KC - 1))
                        nc.vector.tensor_copy(dst[:, g, ts(tt, NT)], ps)

                def ktrans(mt, g):
                    # khT [c, m] -> khp [m, c] on the DMA XBAR (contiguous
                    # staging tile - non-contiguous XBAR dst is broken on HW),
                    # one k-tile (4 m-chunks) per call
                    stage = stp.tile([P, 4, P], bf16, tag="st")
                    nc.sync.dma_start_transpose(stage, khT[:, g, ts(mt, NT)])
                    for j in range(4):
                        for hh in range(2):
                            nc.vector.tensor_copy(
                                khp[:, 4 * mt + j, g, hh * CH:hh * CH + HD],
                                stage[:, j, ts(hh, HD)])

                # ---- out-projection strip (row chunk lc, column tile jt) ----
                def outproj(lc, jt, use_scalar=False):
                    po = psP.tile([P, NT], f32, tag="ps")
                    for cc in range(G):
                        nc.tensor.matmul(po, lhsT=xu[:, cc, ts(lc, P)],
                                         rhs=wo_sb[:, cc, ts(jt, NT)],
                                         start=(cc == 0), stop=(cc == G - 1))
                    ot = opool.tile([P, NT], f32, tag="ot")
                    if use_scalar:
                        nc.scalar.copy(out=ot, in_=po)
                    else:
                        nc.vector.tensor_copy(ot, po)
                    nc.sync.dma_start(out[ts(lc, P), ts(jt, NT)], ot)

                # ---- DMAs: k-side on the sync queue, q-side on the scalar
                # queue so the two streams load concurrently and the prologue
                # projections start ~4us in
                def load_tile_q(src_ap, tt):
                    # q tiles ride the otherwise-empty gpsimd (SWDGE) queue
                    in_t = io.tile([P, KC, NT], bf16, tag="io")
                    nc.gpsimd.dma_start(in_t, src_ap[tt])
                    return in_t

                kt0 = load_tile(kT, 0)
                nc.scalar.dma_start(wq_sb, wqT[:, :])
                qt0 = load_tile_q(qT, 0)
                kt1 = load_tile(kT, 1)
                kt2 = load_tile(kT, 2)
                kt3 = load_tile(kT, 3)
                qt1 = load_tile_q(qT, 1)
                qt2 = load_tile_q(qT, 2)
                qt3 = load_tile_q(qT, 3)
                nc.scalar.dma_start(wo_sb, woT[:, :])

                # ---- prologue: the minimum compute before attention starts
                st = {}
                proj_half(khT, wk_sb, kt0, 0, 0, 0, st)
                proj_half(khT, wk_sb, kt0, 0, 0, 1, st)
                ktrans(0, 0)
                st = {}
                proj_half(qhT, wq_sb, qt0, 0, 0, 0, st)
                proj_half(qhT, wq_sb, qt0, 0, 0, 1, st)

                # ---- filler atoms, ordered by first use ----
                fillers = []

                def add_proj_atoms(dst, w_sb, in_t, tt, g):
                    st = {}
                    fillers.append(lambda st=st: proj_half(dst, w_sb, in_t, tt, g, 0, st))
                    fillers.append(lambda st=st: proj_half(dst, w_sb, in_t, tt, g, 1, st))

                add_proj_atoms(khT, wk_sb, kt1, 1, 0)
                fillers.append(lambda: ktrans(1, 0))
                add_proj_atoms(khT, wk_sb, kt2, 2, 0)
                fillers.append(lambda: ktrans(2, 0))
                add_proj_atoms(khT, wk_sb, kt3, 3, 0)
                fillers.append(lambda: ktrans(3, 0))
                add_proj_atoms(qhT, wq_sb, qt1, 1, 0)
                add_proj_atoms(khT, wk_sb, kt0, 0, 1)
                fillers.append(lambda: ktrans(0, 1))
                add_proj_atoms(khT, wk_sb, kt1, 1, 1)
                fillers.append(lambda: ktrans(1, 1))
                add_proj_atoms(khT, wk_sb, kt2, 2, 1)
                fillers.append(lambda: ktrans(2, 1))
                add_proj_atoms(khT, wk_sb, kt3, 3, 1)
                fillers.append(lambda: ktrans(3, 1))
                add_proj_atoms(qhT, wq_sb, qt2, 2, 0)
                add_proj_atoms(qhT, wq_sb, qt3, 3, 0)
                add_proj_atoms(qhT, wq_sb, qt0, 0, 1)
                add_proj_atoms(qhT, wq_sb, qt1, 1, 1)
                add_proj_atoms(qhT, wq_sb, qt2, 2, 1)
                add_proj_atoms(qhT, wq_sb, qt3, 3, 1)

                # ---- attention ----
                def emit_sp(g, lsl, mc):
                    sps = psS.tile([P, 2 * NT], f32, tag="s")
                    with tc.high_priority(offset=32):
                        nc.tensor.matmul(sps[:, 0:NT],
                                         lhsT=khT[0:HD, g, ts(mc, P)],
                                         rhs=qhT[0:HD, g, lsl],
                                         start=True, stop=True)
                        nc.tensor.matmul(sps[:, NT:2 * NT],
                                         lhsT=khT[HD:P, g, ts(mc, P)],
                                         rhs=qhT[HD:P, g, lsl],
                                         start=True, stop=True)
                    return sps

                def normalize_atoms(g, lsl, dbcs):
                    # deferred one strip: the partition_broadcast issued at the
                    # end of strip (g,l5) has long completed, so the reciprocal
                    # never waits inside the Vector queue; four small atoms so
                    # the Vector queue never gets a long normalize block
                    atoms = []
                    rts = [None, None]
                    for hh in range(2):
                        def recip(hh=hh):
                            # NB: reciprocal_approx_fast mishandles APs with a
                            # non-zero base partition - always full-tile it
                            rt = pbp.tile([P, NT], f32, tag="pb")
                            rts[hh] = rt
                            nc.vector.reciprocal_approx_fast(
                                out=rt, in_=dbcs[hh])
                        def mul(hh=hh):
                            nc.vector.tensor_mul(xu[ts(hh, HD), g, lsl],
                                                 xu[ts(hh, HD), g, lsl],
                                                 rts[hh][ts(hh, HD)])
                        atoms.append((recip, mul))
                    # recips first (independent), then the multiplies
                    return [atoms[0][0], atoms[1][0], atoms[0][1], atoms[1][1]]

                for g in range(G):
                    for l5 in range(L5):
                        lsl = ts(l5, NT)

                        xpsA = psX.tile([CH, NT], f32, tag="x")
                        xpsB = psX.tile([CH, NT], f32, tag="x")
                        sq = [emit_sp(g, lsl, 0)]
                        if MG > 1:
                            sq.append(emit_sp(g, lsl, 1))
                        for mc in range(MG):
                            if mc + 2 < MG:
                                sq.append(emit_sp(g, lsl, mc + 2))
                            es = es_pool.tile([P, 2 * NT], bf16, tag="es")
                            nc.scalar.activation(es, sq.pop(0), Exp, scale=0.125)
                            if fillers:
                                fillers.pop(0)()
                            nc.tensor.matmul(xpsA, lhsT=khp[:, mc, g, 0:CH],
                                             rhs=es[:, 0:NT],
                                             start=(mc == 0), stop=(mc == MG - 1))
                            nc.tensor.matmul(xpsB, lhsT=khp[:, mc, g, CH:2 * CH],
                                             rhs=es[:, NT:2 * NT],
                                             start=(mc == 0), stop=(mc == MG - 1))

                        last_strip = (g == 1 and l5 == L5 - 1)
                        dbcs = []
                        for hh, xps in ((0, xpsA), (1, xpsB)):
                            if last_strip:
                                # ScalarE is idle once the exps are done; use
                                # it so the final den chain never queues
                                nc.scalar.copy(out=xu[ts(hh, HD), g, lsl],
                                               in_=xps[0:HD])
                            else:
                                nc.vector.tensor_copy(xu[ts(hh, HD), g, lsl],
                                                      xps[0:HD])
                            dstg = dstp.tile([1, NT], f32, tag="dst")
                            nc.vector.tensor_copy(dstg, xps[HD:CH])
                            dbc = pbp.tile([P, NT], f32, tag="pb")
                            nc.gpsimd.partition_broadcast(dbc, dstg)
                            dbcs.append(dbc)
                        # reciprocal + multiply deferred one strip
                        for i, atom in enumerate(normalize_atoms(g, lsl, dbcs)):
                            fillers.insert(i, atom)

                        if g == 1:
                            # out-projection for this strip becomes filler work
                            # paced through the next strip; it sits after this
                            # strip's normalize atom, so it reads normalized xu
                            for lc in range(l5 * (NT // P), (l5 + 1) * (NT // P)):
                                for jt in range(JT):
                                    fillers.append(
                                        lambda lc=lc, jt=jt, us=last_strip:
                                            outproj(lc, jt, us))

                while fillers:
                    fillers.pop(0)()

    nc.finalize()
    return nc


def _get_nc(L, M):
    key = (L, M)
    if key not in _cache:
        _cache[key] = _build(L, M)
    return _cache[key]


# head-major channel permutation: new channel c = h*64+d <- original column d*16+h
_PERM = np.array([(c % HD) * NH + c // HD for c in range(DIM)])

last_exec_time_ns = None
last_results = None


def kernel(q, k, v, Wq, Wk, Wv, Wo):  # noqa: ARG001 - v/Wv dead in reference
    global last_exec_time_ns, last_results
    q = np.asarray(q, np.float32)
    k = np.asarray(k, np.float32)
    Wq = np.asarray(Wq, np.float32)
    Wk = np.asarray(Wk, np.float32)
    Wo = np.asarray(Wo, np.float32)
    B, L, _ = q.shape
    M = k.shape[1]

    import ml_dtypes
    bf = ml_dtypes.bfloat16
    Wq_p = Wq[_PERM]            # (1024, 1024) head-major rows
    Wk_p = Wk[_PERM]
    WoT_p = Wo[:, _PERM].T      # (1024 c, 1024 j)

    NT, LT, MT = 512, L // 512, M // 512

    def tiles4(xT):             # [DIM, L] -> [LT, P, KC, NT] tile-contiguous
        return np.ascontiguousarray(
            xT.reshape(KC, P, LT, NT).transpose(2, 1, 0, 3)).astype(bf)

    def wprep(w):               # [DIM, CW] -> [P, KC, CW]
        return np.ascontiguousarray(
            w.reshape(KC, P, CW).transpose(1, 0, 2)).astype(bf)

    qT = [tiles4(q[b].T) for b in range(B)]
    kT = [tiles4(k[b].T) for b in range(B)]
    wqT = [wprep(Wq_p[hg * CW:(hg + 1) * CW, :].T) for hg in range(4)]
    wkT = [wprep(Wk_p[hg * CW:(hg + 1) * CW, :].T) for hg in range(4)]
    woT = [np.ascontiguousarray(
        WoT_p[hg * CW:(hg + 1) * CW, :].reshape(2, P, DIM).transpose(1, 0, 2)
    ).astype(bf) for hg in range(4)]

    in_maps = []
    for core in range(8):
        b, hg = divmod(core, 4)
        in_maps.append({"qT": qT[b], "kT": kT[b], "wqT": wqT[hg],
                        "wkT": wkT[hg], "woT": woT[hg]})

    nc = _get_nc(L, M)
    trace = bool(int(os.environ.get("MHA_TRACE", "0")))
    res = run_bass_kernel_spmd(nc, in_maps, core_ids=list(range(8)), trace=trace)
    last_results = res
    last_exec_time_ns = res.exec_time_ns

    out = np.zeros((B, L, DIM), np.float32)
    for core in range(8):
        b = core // 4
        out[b] += res.results[core]["out"]
    return out
